# revision 20
# baseline (speedup 1.0000x reference)
"""GAT layer kernel v7 for 8 TRN2 NeuronCores (Bass/Tile).

Layout (unchanged from v3): the host lays out a per-edge-slot duplicated h
(dst-grouped, partition-aligned) so the device computes z and the attention
logits for every edge slot with streaming matmuls -- no gather, no
collective.  dst nodes are dealt to cores by total-degree rank %% 8 and
sorted by degree, so superblocks have near-uniform edge counts; runs of
superblocks with equal width W form "classes" that all batched ops use.

v7 changes vs the v3 baseline (136.9us -> ~90-110us measured):

1. h_dup streams as fp8 e3m4 instead of bf16 (halves the dominant DMA
   traffic, 30.3MB -> 15.2MB/core).  Accuracy is preserved by GPFQ-style
   shaped rounding on the host: features are quantized in sequence with
   the running quantization error projected onto the remaining features
   through the device's own rhs matrix [W|u|v] (rel err ~1.2-1.3% vs
   ~4.5%% for round-to-nearest e4m3; gate is 2e-2).  h is scaled x2 into
   the e3m4 grid; the rhs carries the /2.
2. s_src is computed by separate 1-wide matmuls into a contiguous per-sb
   PSUM row (plus one 1-wide dst matmul for s_dst), so the old strided
   per-group s extraction (~24us of small ACT ops) becomes one contiguous
   [P, W] copy per superblock.  z matmuls are 64-wide, 8 per PSUM bank.
3. The per-superblock softmax reduce (tensor_reduce at 1x, ~65us DVE) is
   replaced by per-class pairwise fold-adds at the DVE 2x packed rate,
   and the weight multiply is one batched 4D op per class.
4. PSUM->SBUF z copies alternate ACT/DVE (KCP) to balance engines.

All arithmetic involving h runs on device; host work is layout plus
input quantization.  Superblock widths are rounded to even (not x4).
"""

import os
import numpy as np
import ml_dtypes
from contextlib import ExitStack

import concourse.bass as bass
import concourse.tile as tile
from concourse import bacc, mybir
from concourse.bass_utils import run_bass_kernel_spmd

NCORES = 8
FD = 128   # node feature dim
ZD = 64    # output feature dim
P = 128    # partitions / superblock

BF16 = mybir.dt.bfloat16
F32 = mybir.dt.float32
F8 = mybir.dt.float8e3

LAST_RESULT = None
NEG = -3.0e38


# ----------------------------------------------------------------- host prep

def _prep(src, dst, n_nodes):
    N = n_nodes
    assert N % NCORES == 0
    nsh = N // NCORES
    nsb = (nsh + P - 1) // P
    npad = nsb * P

    deg_tot = np.bincount(dst, minlength=N).astype(np.int64)
    order_tot = np.argsort(-deg_tot, kind="stable")
    rank = np.empty(N, np.int64)
    rank[order_tot] = np.arange(N)
    core_of = (rank % NCORES).astype(np.int64)

    # per-core node order (by degree desc), position -> (sb, p)
    nodes_by_core = []
    pos = np.empty(N, np.int64)           # position of node within its core
    for c in range(NCORES):
        nodes_c = np.flatnonzero(core_of == c)
        o = np.argsort(-deg_tot[nodes_c], kind="stable")
        nodes_c = nodes_c[o]
        nodes_by_core.append(nodes_c)
        pos[nodes_c] = np.arange(len(nodes_c))

    # per-core superblock widths (shared W_sb so one program fits all cores)
    degs_at = np.zeros((NCORES, npad), np.int64)
    for c in range(NCORES):
        degs_at[c, :nsh] = deg_tot[nodes_by_core[c]]
    maxdeg = degs_at.reshape(NCORES, nsb, P).max(axis=(0, 2))
    W = 2 * ((maxdeg + 1) // 2)           # edge cols per superblock (even)
    W = np.maximum(W, 2)
    colbase = np.zeros(nsb + 1, np.int64)
    colbase[1:] = np.cumsum(W)
    ncols = int(colbase[-1])

    # edge -> slot
    d_e = dst
    c_e = core_of[d_e]
    pos_e = pos[d_e]
    eo = np.lexsort((pos_e, c_e))          # stable rank within dst
    c_s, pos_s, src_s = c_e[eo], pos_e[eo], src[eo].astype(np.int64)
    gid = c_s * nsh + pos_s
    j = np.arange(len(gid)) - np.searchsorted(gid, gid, side="left")
    sb_s = pos_s // P
    p_s = pos_s % P
    col_s = colbase[sb_s] + j
    assert (j < W[sb_s]).all()

    return {
        "N": N, "nsh": nsh, "nsb": nsb, "npad": npad,
        "W": W, "colbase": colbase, "ncols": ncols,
        "nodes_by_core": nodes_by_core,
        "c_s": c_s, "p_s": p_s, "sb_s": sb_s, "col_s": col_s, "src_s": src_s,
    }


def _gpfq_quantize(h, R66, scale=2.0, lam=2.0):
    f8 = ml_dtypes.float8_e3m4
    Wt = R66.copy()
    Wt[:, ZD:] *= lam
    G = (Wt * Wt).sum(axis=1)
    hq = np.empty(h.shape, f8)
    r = np.zeros((h.shape[0], Wt.shape[1]), np.float32)
    for i in range(h.shape[1]):
        wi = Wt[i]
        t = (h[:, i] + (r @ wi) / G[i]) * scale
        np.clip(t, -15.0, 15.0, out=t)
        q = t.astype(f8)
        hq[:, i] = q
        r += np.outer(h[:, i] - q.astype(np.float32) / scale, wi)
    return hq


def _host_inputs(h, W_fc, W_attn, meta):
    nsh, nsb, npad = meta["nsh"], meta["nsb"], meta["npad"]
    W, colbase, ncols = meta["W"], meta["colbase"], meta["ncols"]
    bf16 = ml_dtypes.bfloat16

    nblocks = int(nsb + ncols)            # per sb: 1 dst block + W[sb] blocks
    blockbase = np.zeros(nsb, np.int64)
    np.cumsum(1 + W[:-1], out=blockbase[1:]) if nsb > 1 else None

    wft = np.ascontiguousarray(W_fc.T.astype(np.float32))   # [64, 128]
    wzb = np.ascontiguousarray((W_fc * 0.5).astype(bf16))   # [128, 64]
    wa2 = np.ascontiguousarray(
        np.stack([W_attn[:ZD, 0], W_attn[ZD:, 0]], axis=1).astype(np.float32))

    u = W_fc @ W_attn[:ZD]
    v = W_fc @ W_attn[ZD:]
    R66 = np.concatenate(
        [W_fc.astype(bf16).astype(np.float32),
         u.astype(bf16).astype(np.float32),
         v.astype(bf16).astype(np.float32)], axis=1)
    hq = _gpfq_quantize(h.astype(np.float32), R66)
    hT = np.ascontiguousarray(hq.T)       # [128, N] f8 (x2 scaled)

    # slot -> source node (global), -1 = pad
    c_s, p_s, sb_s, col_s, src_s = (meta["c_s"], meta["p_s"], meta["sb_s"],
                                    meta["col_s"], meta["src_s"])

    in_maps = []
    for c in range(NCORES):
        # h_dup: [128, nblocks*128] bf16
        srcmat = np.full((nblocks, P), -1, np.int64)
        # dst blocks
        nodes_c = meta["nodes_by_core"][c]
        dst_mat = np.full((nsb, P), -1, np.int64)
        dst_mat.reshape(-1)[:nsh] = nodes_c
        srcmat[blockbase] = dst_mat
        # edge blocks
        sel = c_s == c
        blk = blockbase[sb_s[sel]] + 1 + (col_s[sel] - colbase[sb_s[sel]])
        srcmat[blk, p_s[sel]] = src_s[sel]

        flat = srcmat.reshape(-1)
        hd = np.zeros((FD, nblocks * P), ml_dtypes.float8_e3m4)
        valid = flat >= 0
        hd[:, valid] = hT[:, flat[valid]]

        # mask: [128, ncols] bf16, 1 where edge exists else 0
        mask = np.zeros((P, ncols), bf16)
        mask[p_s[sel], col_s[sel]] = 1.0
        in_maps.append({
            "hdup": np.ascontiguousarray(hd),
            "mask": np.ascontiguousarray(mask),
            "WfT": wft, "Wzb": wzb, "Wa2": wa2,
        })
    return in_maps, nblocks


# ------------------------------------------------------------- device build

def _build_program(meta, nblocks):
    nsb, npad, ncols = meta["nsb"], meta["npad"], meta["ncols"]
    W, colbase = meta["W"], meta["colbase"]
    blockbase = np.zeros(nsb, np.int64)
    if nsb > 1:
        np.cumsum(1 + W[:-1], out=blockbase[1:])

    GS = 8                                 # z-psum group: 8 * 64 fp32 = 1 bank

    # classes = runs of superblocks with equal width; phases batch per class
    classes = []
    s0 = 0
    for s in range(1, nsb + 1):
        if s == nsb or W[s] != W[s0]:
            classes.append((s0, s - s0, int(W[s0])))
            s0 = s

    ndev = int(os.environ.get("KNC", str(NCORES)))
    nc = bacc.Bacc("TRN2", target_bir_lowering=False, debug=False,
                   enable_asserts=False, num_devices=ndev)

    hdup_t = nc.dram_tensor("hdup", [FD, nblocks * P], F8,
                            kind="ExternalInput")
    mask_t = nc.dram_tensor("mask", [P, ncols], BF16,
                            kind="ExternalInput")
    WfT_t = nc.dram_tensor("WfT", [ZD, FD], F32, kind="ExternalInput")
    Wzb_t = nc.dram_tensor("Wzb", [FD, ZD], BF16, kind="ExternalInput")
    Wa2_t = nc.dram_tensor("Wa2", [ZD, 2], F32, kind="ExternalInput")
    out_t = nc.dram_tensor("out", [npad, ZD], F32, kind="ExternalOutput")

    KREP = int(os.environ.get("KREP", "1"))
    KCP = int(os.environ.get("KCP", "4"))   # every KCP-th z-copy goes to DVE
    A = mybir.AluOpType

    with tile.TileContext(nc) as tc, ExitStack() as ctx:
        wpool = ctx.enter_context(tc.tile_pool(name="w", bufs=1))
        ppool = ctx.enter_context(tc.tile_pool(name="ps", bufs=1,
                                               space="PSUM"))
        sppool = ctx.enter_context(tc.tile_pool(name="sps", bufs=2,
                                                space="PSUM"))
        zppool = ctx.enter_context(tc.tile_pool(name="zps", bufs=5,
                                                space="PSUM"))
        rpool = ctx.enter_context(tc.tile_pool(name="res", bufs=1))

        # ---- weights: rhs66 = [W/2 | u/2 | v/2] bf16 ---------------------
        wft = wpool.tile([ZD, FD], F32)
        nc.sync.dma_start(wft[:], WfT_t.ap())
        wa2 = wpool.tile([ZD, 2], F32)
        nc.sync.dma_start(wa2[:], Wa2_t.ap())
        wzb = wpool.tile([FD, ZD], BF16)
        nc.sync.dma_start(wzb[:], Wzb_t.ap())

        uv_ps = ppool.tile([FD, 2], F32, tag="ups")
        nc.tensor.matmul(uv_ps[:], lhsT=wft[:], rhs=wa2[:],
                         start=True, stop=True)
        rhs66 = wpool.tile([FD, ZD + 2], BF16)
        nc.vector.tensor_copy(rhs66[:, 0:ZD], wzb[:])
        nc.vector.tensor_scalar_mul(rhs66[:, ZD:ZD + 2], uv_ps[:], 0.5)

        maskt = rpool.tile([P, ncols], BF16, tag="mask")
        nc.sync.dma_start(maskt[:], mask_t.ap())

        for _krep in range(KREP):
         with ExitStack() as bctx:
            hpool = bctx.enter_context(tc.tile_pool(name="hld", bufs=4))
            epool = bctx.enter_context(tc.tile_pool(name="e", bufs=2))

            ztf = rpool.tile([P, ZD * ncols], BF16, tag="ztf")
            z3f = ztf[:].rearrange("p (k w) -> p k w", w=ncols)
            ssf = rpool.tile([P, ncols], F32, tag="ssf")
            sdxf = rpool.tile([P, ncols], F32, tag="sdxf")
            w2f = rpool.tile([P, ncols], BF16, tag="w2f")
            nd = rpool.tile([P, nsb * (ZD + 1)], F32, tag="nd")
            nd3 = nd[:].rearrange("p (s k) -> p s k", k=ZD + 1)
            ofin = rpool.tile([P, nsb * ZD], F32, tag="ofin")
            o3 = ofin[:].rearrange("p (s k) -> p s k", k=ZD)

            cpi = 0
            for (cs0, ccnt, Wc) in classes:
                # ---- phase 1: stream h_dup; z matmuls (64-wide) into
                # bank-sized psum groups; s matmuls (1-wide) into a
                # contiguous per-sb psum row; copy z (ACT/DVE) + s to SBUF
                for sb in range(cs0, cs0 + ccnt):
                    nb = 1 + Wc
                    b0 = int(blockbase[sb])
                    c0 = int(colbase[sb])

                    hs = hpool.tile([FD, nb * P], F8, tag="hs")
                    nc.sync.dma_start(
                        hs[:], hdup_t.ap()[:, b0 * P:(b0 + nb) * P])

                    sp = sppool.tile([P, Wc + 1], F32, tag="sp")
                    nc.tensor.matmul(
                        sp[:, Wc:Wc + 1], lhsT=hs[:, 0:P],
                        rhs=rhs66[:, ZD + 1:ZD + 2], start=True, stop=True)
                    nc.scalar.copy(
                        sdxf[:, c0:c0 + Wc],
                        sp[:, Wc:Wc + 1].to_broadcast([P, Wc]))

                    for g0 in range(0, Wc, GS):
                        g1 = min(g0 + GS, Wc)
                        zp = zppool.tile([P, GS * ZD], F32, tag="zps")
                        zp3 = zp[:].rearrange("p (g k) -> p g k", k=ZD)
                        zpt = zp[:].rearrange("p (g k) -> p k g", k=ZD)
                        for b in range(g0, g1):
                            nc.tensor.matmul(
                                zp3[:, b - g0, :],
                                lhsT=hs[:, (1 + b) * P:(2 + b) * P],
                                rhs=rhs66[:, 0:ZD], start=True, stop=True)
                            nc.tensor.matmul(
                                sp[:, b:b + 1],
                                lhsT=hs[:, (1 + b) * P:(2 + b) * P],
                                rhs=rhs66[:, ZD:ZD + 1], start=True,
                                stop=True)
                        cpi += 1
                        if cpi % KCP:
                            nc.scalar.copy(z3f[:, :, c0 + g0:c0 + g1],
                                           zpt[:, 0:ZD, 0:g1 - g0])
                        else:
                            nc.vector.tensor_copy(
                                z3f[:, :, c0 + g0:c0 + g1],
                                zpt[:, 0:ZD, 0:g1 - g0])
                    nc.scalar.copy(ssf[:, c0:c0 + Wc], sp[:, 0:Wc])

                # ---- phase 2: softmax weights for the whole class --------
                cc0 = int(colbase[cs0])
                cc1 = int(colbase[cs0 + ccnt])
                cw = cc1 - cc0
                elog = epool.tile([P, cw], F32, tag="elog")
                nc.gpsimd.tensor_tensor(
                    out=elog[:], in0=ssf[:, cc0:cc1], in1=sdxf[:, cc0:cc1],
                    op=A.add)
                nc.vector.scalar_tensor_tensor(
                    elog[:], elog[:], 0.01, elog[:], A.mult, A.max)
                wch = epool.tile([P, cw], BF16, tag="wch")
                nc.scalar.activation(wch[:], elog[:],
                                     mybir.ActivationFunctionType.Exp)
                nc.gpsimd.tensor_tensor(
                    out=w2f[:, cc0:cc1], in0=wch[:], in1=maskt[:, cc0:cc1],
                    op=A.mult)

                # ---- phase 3: weighted fold-reduce for the class ---------
                zcl = (z3f[:, :, cc0:cc1]
                       .rearrange("p k (s c) -> p k s c", c=Wc))
                wcl = (w2f[:, cc0:cc1]
                       .rearrange("p (s c) -> p s c", c=Wc))
                nc.vector.tensor_tensor(
                    out=zcl, in0=zcl,
                    in1=wcl.unsqueeze(1).to_broadcast([P, ZD, ccnt, Wc]),
                    op=A.mult)
                n = Wc
                while n > 2:
                    if n % 2:
                        nc.vector.tensor_tensor(
                            out=zcl[:, :, :, 0:1], in0=zcl[:, :, :, 0:1],
                            in1=zcl[:, :, :, n - 1:n], op=A.add)
                        n -= 1
                    half = n // 2
                    nc.vector.tensor_tensor(
                        out=zcl[:, :, :, 0:half], in0=zcl[:, :, :, 0:half],
                        in1=zcl[:, :, :, half:n], op=A.add)
                    n = half
                ndv = (nd3[:, cs0:cs0 + ccnt, 0:ZD]
                       .rearrange("p s k -> p k s"))
                if n == 2:
                    nc.vector.tensor_tensor(
                        out=ndv, in0=zcl[:, :, :, 0], in1=zcl[:, :, :, 1],
                        op=A.add)
                else:
                    nc.vector.tensor_copy(ndv, zcl[:, :, :, 0])
                nc.vector.tensor_reduce(
                    out=nd3[:, cs0:cs0 + ccnt, ZD], in_=wcl,
                    axis=mybir.AxisListType.X, op=A.add)

            # ---- tail: batched divide + output --------------------------
            deng = epool.tile([P, nsb], F32, tag="deng")
            nc.vector.tensor_scalar_max(deng[:], nd3[:, :, ZD], 1e-30)
            rcp = epool.tile([P, nsb], F32, tag="rcp")
            nc.vector.reciprocal(rcp[:], deng[:])
            nc.gpsimd.tensor_tensor(
                out=o3[:], in0=nd3[:, :, 0:ZD],
                in1=rcp[:].unsqueeze(2).to_broadcast([P, nsb, ZD]),
                op=A.mult)
            nc.sync.dma_start(
                out_t.ap().rearrange("(s p) c -> p s c", p=P), o3)

    nc.compile()
    return nc


# ------------------------------------------------------------------- driver

def kernel(h, src, dst, W_fc, W_attn):
    global LAST_RESULT
    h = np.asarray(h, np.float32)
    src = np.asarray(src, np.int32)
    dst = np.asarray(dst, np.int32)
    W_fc = np.asarray(W_fc, np.float32)
    W_attn = np.asarray(W_attn, np.float32)
    N = h.shape[0]

    meta = _prep(src, dst, N)
    in_maps, nblocks = _host_inputs(h, W_fc, W_attn, meta)
    nc = _build_program(meta, nblocks)

    res = run_bass_kernel_spmd(nc, in_maps, core_ids=list(range(NCORES)))
    LAST_RESULT = res

    nsh = meta["nsh"]
    out = np.zeros((N, ZD), np.float32)
    for c in range(NCORES):
        out[meta["nodes_by_core"][c]] = res.results[c]["out"][:nsh]
    return out



# revision 21
# speedup vs baseline: 1.0865x; 1.0865x over previous
"""GAT layer kernel v7 for 8 TRN2 NeuronCores (Bass/Tile).

Layout (unchanged from v3): the host lays out a per-edge-slot duplicated h
(dst-grouped, partition-aligned) so the device computes z and the attention
logits for every edge slot with streaming matmuls -- no gather, no
collective.  dst nodes are dealt to cores by total-degree rank %% 8 and
sorted by degree, so superblocks have near-uniform edge counts; runs of
superblocks with equal width W form "classes" that all batched ops use.

v7 changes vs the v3 baseline (136.9us -> ~90-110us measured):

1. h_dup streams as fp8 e3m4 instead of bf16 (halves the dominant DMA
   traffic, 30.3MB -> 15.2MB/core).  Accuracy is preserved by GPFQ-style
   shaped rounding on the host: features are quantized in sequence with
   the running quantization error projected onto the remaining features
   through the device's own rhs matrix [W|u|v] (rel err ~1.2-1.3% vs
   ~4.5%% for round-to-nearest e4m3; gate is 2e-2).  h is scaled x2 into
   the e3m4 grid; the rhs carries the /2.
2. s_src is computed by separate 1-wide matmuls into a contiguous per-sb
   PSUM row (plus one 1-wide dst matmul for s_dst), so the old strided
   per-group s extraction (~24us of small ACT ops) becomes one contiguous
   [P, W] copy per superblock.  z matmuls are 64-wide, 8 per PSUM bank.
3. The per-superblock softmax reduce (tensor_reduce at 1x, ~65us DVE) is
   replaced by per-class pairwise fold-adds at the DVE 2x packed rate,
   and the weight multiply is one batched 4D op per class.
4. PSUM->SBUF z copies alternate ACT/DVE (KCP) to balance engines.

All arithmetic involving h runs on device; host work is layout plus
input quantization.  Superblock widths are rounded to even (not x4).
"""

import os
import numpy as np
import ml_dtypes
from contextlib import ExitStack

import concourse.bass as bass
import concourse.tile as tile
from concourse import bacc, mybir
from concourse.bass_utils import run_bass_kernel_spmd

NCORES = 8
FD = 128   # node feature dim
ZD = 64    # output feature dim
P = 128    # partitions / superblock

BF16 = mybir.dt.bfloat16
F32 = mybir.dt.float32
F8 = mybir.dt.float8e3

LAST_RESULT = None
NEG = -3.0e38


# ----------------------------------------------------------------- host prep

def _prep(src, dst, n_nodes):
    N = n_nodes
    assert N % NCORES == 0
    nsh = N // NCORES
    nsb = (nsh + P - 1) // P
    npad = nsb * P

    deg_tot = np.bincount(dst, minlength=N).astype(np.int64)
    order_tot = np.argsort(-deg_tot, kind="stable")
    rank = np.empty(N, np.int64)
    rank[order_tot] = np.arange(N)
    core_of = (rank % NCORES).astype(np.int64)

    # per-core node order (by degree desc), position -> (sb, p)
    nodes_by_core = []
    pos = np.empty(N, np.int64)           # position of node within its core
    for c in range(NCORES):
        nodes_c = np.flatnonzero(core_of == c)
        o = np.argsort(-deg_tot[nodes_c], kind="stable")
        nodes_c = nodes_c[o]
        nodes_by_core.append(nodes_c)
        pos[nodes_c] = np.arange(len(nodes_c))

    # per-core superblock widths (shared W_sb so one program fits all cores)
    degs_at = np.zeros((NCORES, npad), np.int64)
    for c in range(NCORES):
        degs_at[c, :nsh] = deg_tot[nodes_by_core[c]]
    maxdeg = degs_at.reshape(NCORES, nsb, P).max(axis=(0, 2))
    W = 2 * ((maxdeg + 1) // 2)           # edge cols per superblock (even)
    W = np.maximum(W, 2)
    colbase = np.zeros(nsb + 1, np.int64)
    colbase[1:] = np.cumsum(W)
    ncols = int(colbase[-1])

    # edge -> slot
    d_e = dst
    c_e = core_of[d_e]
    pos_e = pos[d_e]
    eo = np.lexsort((pos_e, c_e))          # stable rank within dst
    c_s, pos_s, src_s = c_e[eo], pos_e[eo], src[eo].astype(np.int64)
    gid = c_s * nsh + pos_s
    j = np.arange(len(gid)) - np.searchsorted(gid, gid, side="left")
    sb_s = pos_s // P
    p_s = pos_s % P
    col_s = colbase[sb_s] + j
    assert (j < W[sb_s]).all()

    return {
        "N": N, "nsh": nsh, "nsb": nsb, "npad": npad,
        "W": W, "colbase": colbase, "ncols": ncols,
        "nodes_by_core": nodes_by_core,
        "c_s": c_s, "p_s": p_s, "sb_s": sb_s, "col_s": col_s, "src_s": src_s,
    }


def _gpfq_quantize(h, R66, scale=2.0, lam=2.0):
    f8 = ml_dtypes.float8_e3m4
    Wt = R66.copy()
    Wt[:, ZD:] *= lam
    G = (Wt * Wt).sum(axis=1)
    hq = np.empty(h.shape, f8)
    r = np.zeros((h.shape[0], Wt.shape[1]), np.float32)
    for i in range(h.shape[1]):
        wi = Wt[i]
        t = (h[:, i] + (r @ wi) / G[i]) * scale
        np.clip(t, -15.0, 15.0, out=t)
        q = t.astype(f8)
        hq[:, i] = q
        r += np.outer(h[:, i] - q.astype(np.float32) / scale, wi)
    return hq


def _host_inputs(h, W_fc, W_attn, meta):
    nsh, nsb, npad = meta["nsh"], meta["nsb"], meta["npad"]
    W, colbase, ncols = meta["W"], meta["colbase"], meta["ncols"]
    bf16 = ml_dtypes.bfloat16

    nblocks = int(nsb + ncols)            # per sb: 1 dst block + W[sb] blocks
    blockbase = np.zeros(nsb, np.int64)
    np.cumsum(1 + W[:-1], out=blockbase[1:]) if nsb > 1 else None

    wft = np.ascontiguousarray(W_fc.T.astype(np.float32))   # [64, 128]
    wzb = np.ascontiguousarray((W_fc * 0.5).astype(bf16))   # [128, 64]
    wa2 = np.ascontiguousarray(
        np.stack([W_attn[:ZD, 0], W_attn[ZD:, 0]], axis=1).astype(np.float32))

    u = W_fc @ W_attn[:ZD]
    v = W_fc @ W_attn[ZD:]
    R66 = np.concatenate(
        [W_fc.astype(bf16).astype(np.float32),
         u.astype(bf16).astype(np.float32),
         v.astype(bf16).astype(np.float32)], axis=1)
    hq = _gpfq_quantize(h.astype(np.float32), R66)
    hT = np.ascontiguousarray(hq.T)       # [128, N] f8 (x2 scaled)

    # slot -> source node (global), -1 = pad
    c_s, p_s, sb_s, col_s, src_s = (meta["c_s"], meta["p_s"], meta["sb_s"],
                                    meta["col_s"], meta["src_s"])

    in_maps = []
    for c in range(NCORES):
        # h_dup: [128, nblocks*128] bf16
        srcmat = np.full((nblocks, P), -1, np.int64)
        # dst blocks
        nodes_c = meta["nodes_by_core"][c]
        dst_mat = np.full((nsb, P), -1, np.int64)
        dst_mat.reshape(-1)[:nsh] = nodes_c
        srcmat[blockbase] = dst_mat
        # edge blocks
        sel = c_s == c
        blk = blockbase[sb_s[sel]] + 1 + (col_s[sel] - colbase[sb_s[sel]])
        srcmat[blk, p_s[sel]] = src_s[sel]

        flat = srcmat.reshape(-1)
        hd = np.zeros((FD, nblocks * P), ml_dtypes.float8_e3m4)
        valid = flat >= 0
        hd[:, valid] = hT[:, flat[valid]]

        # mask: [128, ncols] bf16, 1 where edge exists else 0
        mask = np.zeros((P, ncols), bf16)
        mask[p_s[sel], col_s[sel]] = 1.0
        in_maps.append({
            "hdup": np.ascontiguousarray(hd),
            "mask": np.ascontiguousarray(mask),
            "WfT": wft, "Wzb": wzb, "Wa2": wa2,
        })
    return in_maps, nblocks


# ------------------------------------------------------------- device build

def _build_program(meta, nblocks):
    nsb, npad, ncols = meta["nsb"], meta["npad"], meta["ncols"]
    W, colbase = meta["W"], meta["colbase"]
    blockbase = np.zeros(nsb, np.int64)
    if nsb > 1:
        np.cumsum(1 + W[:-1], out=blockbase[1:])

    GS = 8                                 # z-psum group: 8 * 64 fp32 = 1 bank

    # classes = runs of superblocks with equal width; phases batch per class
    classes = []
    s0 = 0
    for s in range(1, nsb + 1):
        if s == nsb or W[s] != W[s0]:
            classes.append((s0, s - s0, int(W[s0])))
            s0 = s

    ndev = int(os.environ.get("KNC", str(NCORES)))
    nc = bacc.Bacc("TRN2", target_bir_lowering=False, debug=False,
                   enable_asserts=False, num_devices=ndev)

    hdup_t = nc.dram_tensor("hdup", [FD, nblocks * P], F8,
                            kind="ExternalInput")
    mask_t = nc.dram_tensor("mask", [P, ncols], BF16,
                            kind="ExternalInput")
    WfT_t = nc.dram_tensor("WfT", [ZD, FD], F32, kind="ExternalInput")
    Wzb_t = nc.dram_tensor("Wzb", [FD, ZD], BF16, kind="ExternalInput")
    Wa2_t = nc.dram_tensor("Wa2", [ZD, 2], F32, kind="ExternalInput")
    out_t = nc.dram_tensor("out", [npad, ZD], F32, kind="ExternalOutput")

    KREP = int(os.environ.get("KREP", "1"))
    KCP = int(os.environ.get("KCP", "4"))   # every KCP-th z-copy goes to DVE
    A = mybir.AluOpType

    with tile.TileContext(nc) as tc, ExitStack() as ctx:
        wpool = ctx.enter_context(tc.tile_pool(name="w", bufs=1))
        ppool = ctx.enter_context(tc.tile_pool(name="ps", bufs=1,
                                               space="PSUM"))
        sppool = ctx.enter_context(tc.tile_pool(name="sps", bufs=2,
                                                space="PSUM"))
        zppool = ctx.enter_context(tc.tile_pool(name="zps", bufs=5,
                                                space="PSUM"))
        rpool = ctx.enter_context(tc.tile_pool(name="res", bufs=1))

        # ---- weights: rhs66 = [W/2 | u/2 | v/2] bf16 ---------------------
        wft = wpool.tile([ZD, FD], F32)
        nc.sync.dma_start(wft[:], WfT_t.ap())
        wa2 = wpool.tile([ZD, 2], F32)
        nc.sync.dma_start(wa2[:], Wa2_t.ap())
        wzb = wpool.tile([FD, ZD], BF16)
        nc.sync.dma_start(wzb[:], Wzb_t.ap())

        uv_ps = ppool.tile([FD, 2], F32, tag="ups")
        nc.tensor.matmul(uv_ps[:], lhsT=wft[:], rhs=wa2[:],
                         start=True, stop=True)
        rhs66 = wpool.tile([FD, ZD + 2], BF16)
        nc.vector.tensor_copy(rhs66[:, 0:ZD], wzb[:])
        nc.vector.tensor_scalar_mul(rhs66[:, ZD:ZD + 2], uv_ps[:], 0.5)

        maskt = rpool.tile([P, ncols], BF16, tag="mask")
        nc.sync.dma_start(maskt[:], mask_t.ap())

        for _krep in range(KREP):
         with ExitStack() as bctx:
            hpool = bctx.enter_context(tc.tile_pool(name="hld", bufs=4))
            epool = bctx.enter_context(tc.tile_pool(name="e", bufs=2))

            ztf = rpool.tile([P, ZD * ncols], BF16, tag="ztf")
            z3f = ztf[:].rearrange("p (k w) -> p k w", w=ncols)
            ssf = rpool.tile([P, ncols], F32, tag="ssf")
            sdxf = rpool.tile([P, ncols], F32, tag="sdxf")
            w2f = rpool.tile([P, ncols], BF16, tag="w2f")
            nd = rpool.tile([P, nsb * (ZD + 1)], F32, tag="nd")
            nd3 = nd[:].rearrange("p (s k) -> p s k", k=ZD + 1)
            ofin = rpool.tile([P, nsb * ZD], F32, tag="ofin")
            o3 = ofin[:].rearrange("p (s k) -> p s k", k=ZD)

            cpi = 0
            for (cs0, ccnt, Wc) in classes:
                # ---- phase 1: stream h_dup; z matmuls (64-wide) into
                # bank-sized psum groups; s matmuls (1-wide) into a
                # contiguous per-sb psum row; copy z (ACT/DVE) + s to SBUF
                for sb in range(cs0, cs0 + ccnt):
                    nb = 1 + Wc
                    b0 = int(blockbase[sb])
                    c0 = int(colbase[sb])

                    hs = hpool.tile([FD, nb * P], F8, tag="hs")
                    nc.sync.dma_start(
                        hs[:], hdup_t.ap()[:, b0 * P:(b0 + nb) * P])

                    sp = sppool.tile([P, Wc + 1], F32, tag="sp")
                    nc.tensor.matmul(
                        sp[:, Wc:Wc + 1], lhsT=hs[:, 0:P],
                        rhs=rhs66[:, ZD + 1:ZD + 2], start=True, stop=True)
                    nc.scalar.copy(
                        sdxf[:, c0:c0 + Wc],
                        sp[:, Wc:Wc + 1].to_broadcast([P, Wc]))

                    for g0 in range(0, Wc, GS):
                        g1 = min(g0 + GS, Wc)
                        zp = zppool.tile([P, GS * ZD], F32, tag="zps")
                        zp3 = zp[:].rearrange("p (g k) -> p g k", k=ZD)
                        zpt = zp[:].rearrange("p (g k) -> p k g", k=ZD)
                        for b in range(g0, g1):
                            nc.tensor.matmul(
                                zp3[:, b - g0, :],
                                lhsT=hs[:, (1 + b) * P:(2 + b) * P],
                                rhs=rhs66[:, 0:ZD], start=True, stop=True)
                            nc.tensor.matmul(
                                sp[:, b:b + 1],
                                lhsT=hs[:, (1 + b) * P:(2 + b) * P],
                                rhs=rhs66[:, ZD:ZD + 1], start=True,
                                stop=True)
                        cpi += 1
                        if cpi % KCP:
                            nc.scalar.copy(z3f[:, :, c0 + g0:c0 + g1],
                                           zpt[:, 0:ZD, 0:g1 - g0])
                        else:
                            nc.vector.tensor_copy(
                                z3f[:, :, c0 + g0:c0 + g1],
                                zpt[:, 0:ZD, 0:g1 - g0])
                    nc.scalar.copy(ssf[:, c0:c0 + Wc], sp[:, 0:Wc])

                # ---- phase 2: softmax weights for the whole class --------
                cc0 = int(colbase[cs0])
                cc1 = int(colbase[cs0 + ccnt])
                cw = cc1 - cc0
                elog = epool.tile([P, cw], F32, tag="elog")
                nc.vector.tensor_tensor(
                    out=elog[:], in0=ssf[:, cc0:cc1], in1=sdxf[:, cc0:cc1],
                    op=A.add)
                nc.vector.scalar_tensor_tensor(
                    elog[:], elog[:], 0.01, elog[:], A.mult, A.max)
                wch = epool.tile([P, cw], BF16, tag="wch")
                nc.scalar.activation(wch[:], elog[:],
                                     mybir.ActivationFunctionType.Exp)
                nc.vector.tensor_tensor(
                    out=w2f[:, cc0:cc1], in0=wch[:], in1=maskt[:, cc0:cc1],
                    op=A.mult)

                # ---- phase 3: weighted fold-reduce for the class ---------
                zcl = (z3f[:, :, cc0:cc1]
                       .rearrange("p k (s c) -> p k s c", c=Wc))
                wcl = (w2f[:, cc0:cc1]
                       .rearrange("p (s c) -> p s c", c=Wc))
                nc.vector.tensor_tensor(
                    out=zcl, in0=zcl,
                    in1=wcl.unsqueeze(1).to_broadcast([P, ZD, ccnt, Wc]),
                    op=A.mult)
                n = Wc
                while n > 2:
                    if n % 2:
                        nc.vector.tensor_tensor(
                            out=zcl[:, :, :, 0:1], in0=zcl[:, :, :, 0:1],
                            in1=zcl[:, :, :, n - 1:n], op=A.add)
                        n -= 1
                    half = n // 2
                    nc.vector.tensor_tensor(
                        out=zcl[:, :, :, 0:half], in0=zcl[:, :, :, 0:half],
                        in1=zcl[:, :, :, half:n], op=A.add)
                    n = half
                ndv = (nd3[:, cs0:cs0 + ccnt, 0:ZD]
                       .rearrange("p s k -> p k s"))
                if n == 2:
                    nc.vector.tensor_tensor(
                        out=ndv, in0=zcl[:, :, :, 0], in1=zcl[:, :, :, 1],
                        op=A.add)
                else:
                    nc.vector.tensor_copy(ndv, zcl[:, :, :, 0])
                nc.vector.tensor_reduce(
                    out=nd3[:, cs0:cs0 + ccnt, ZD], in_=wcl,
                    axis=mybir.AxisListType.X, op=A.add)

            # ---- tail: batched divide + output --------------------------
            deng = epool.tile([P, nsb], F32, tag="deng")
            nc.vector.tensor_scalar_max(deng[:], nd3[:, :, ZD], 1e-30)
            rcp = epool.tile([P, nsb], F32, tag="rcp")
            nc.vector.reciprocal(rcp[:], deng[:])
            nc.gpsimd.tensor_tensor(
                out=o3[:], in0=nd3[:, :, 0:ZD],
                in1=rcp[:].unsqueeze(2).to_broadcast([P, nsb, ZD]),
                op=A.mult)
            nc.sync.dma_start(
                out_t.ap().rearrange("(s p) c -> p s c", p=P), o3)

    nc.compile()
    return nc


# ------------------------------------------------------------------- driver

def kernel(h, src, dst, W_fc, W_attn):
    global LAST_RESULT
    h = np.asarray(h, np.float32)
    src = np.asarray(src, np.int32)
    dst = np.asarray(dst, np.int32)
    W_fc = np.asarray(W_fc, np.float32)
    W_attn = np.asarray(W_attn, np.float32)
    N = h.shape[0]

    meta = _prep(src, dst, N)
    in_maps, nblocks = _host_inputs(h, W_fc, W_attn, meta)
    nc = _build_program(meta, nblocks)

    res = run_bass_kernel_spmd(nc, in_maps, core_ids=list(range(NCORES)))
    LAST_RESULT = res

    nsh = meta["nsh"]
    out = np.zeros((N, ZD), np.float32)
    for c in range(NCORES):
        out[meta["nodes_by_core"][c]] = res.results[c]["out"][:nsh]
    return out

